# revision 3
# baseline (speedup 1.0000x reference)
"""MinGRU (2-layer) Trainium2 Bass kernel, v2.

Problem: h[8,4096,1024] f32, W0/W1 [1024,3072] f32.
Per layer: z = h @ W; hidden,gate,proj = split(z);
  a = sigmoid(-gate); g_hidden = relu(hidden) + min(sigmoid(hidden), 0.5)
  scan: out_t = a_t*out_{t-1} + (1-a_t)*g_hidden_t   (fp32 scan state)
  h' = sigmoid(proj)*out + (1-sigmoid(proj))*h

Sharding: one batch row per core (B=8 over 8 cores), weights replicated.

v2 design vs the 914us baseline:
  - gate/proj matmuls in fp8(e4m3) with DoubleRow perf mode (2 k-tiles per
    instruction, 2x PE throughput); hidden matmul stays fp16 for accuracy
    (the g_hidden path dominates error; measured ~2e-3 rel err in numpy).
  - all elementwise in fp16 SBUF tiles -> DVE 2x/4x perf modes.
  - host pre-transposes h to [H,T] fp16+fp8 (no DMA-transpose, no PE
    transpose); output y is written as [H,T] fp16 and host re-transposes.
  - scan in linear space via DVE tensor_tensor_scan (fp32 internal state).
"""

import sys

if "/opt/trn_rl_repo" not in sys.path:
    sys.path.insert(0, "/opt/trn_rl_repo")

from contextlib import ExitStack

import numpy as np
import ml_dtypes

import concourse.bass as bass
import concourse.tile as tile
from concourse import bacc, mybir
from concourse import bass_utils

T, H, H3 = 4096, 1024, 3072
TC = 512                 # time chunk (= one PSUM bank of fp32)
NCHUNK = T // TC
NFB = H // 128           # output feature blocks
NK = H // 128            # contraction k-tiles
F32 = mybir.dt.float32
F16 = mybir.dt.float16
F8 = mybir.dt.float8e4
ACT = mybir.ActivationFunctionType
ALU = mybir.AluOpType
DR = mybir.MatmulPerfMode.DoubleRow

# Which of (hidden, gate, proj) run in fp8 DoubleRow; the rest run fp16.
FP8_OUT = (False, True, True)
SH = 8.0                 # fp8 activation scale
SW = 32.0                # fp8 weight scale
INV8 = 1.0 / (SH * SW)

N8 = sum(FP8_OUT)
N16 = 3 - N8
C8 = N8 * H              # fp8 weight columns per layer
C16 = N16 * H            # fp16 weight columns per layer
# column offset of each output within its (fp8|fp16) packed weight tensor
_off8, _off16 = 0, 0
OFF = []
for _u8 in FP8_OUT:
    if _u8:
        OFF.append(_off8)
        _off8 += H
    else:
        OFF.append(_off16)
        _off16 += H


def _emit_layer(nc, i, li, f, w16_sb, w8_sb, rhs16, rhs8, dst16, dst8,
                mm_psum, ew, carries, y16):
    """Matmuls + elementwise for one (chunk, layer, feature-block)."""
    psums = []
    for o in range(3):
        p = mm_psum.tile([128, TC], F32, tag=f"p{o}")
        psums.append(p)
        if FP8_OUT[o]:
            w = w8_sb[li]
            base = OFF[o] + f * 128
            for k in range(0, NK, 2):
                nc.tensor.matmul(p[:], w[:, k:k + 2, base:base + 128],
                                 rhs8[:, k:k + 2, :], perf_mode=DR,
                                 start=(k == 0), stop=(k == NK - 2))
        else:
            w = w16_sb[li]
            base = OFF[o] + f * 128
            for k in range(NK):
                nc.tensor.matmul(p[:], w[:, k, base:base + 128],
                                 rhs16[:, k, :],
                                 start=(k == 0), stop=(k == NK - 1))
    ph, pg, pp = psums
    sc_h = INV8 if FP8_OUT[0] else 1.0
    sc_g = INV8 if FP8_OUT[1] else 1.0
    sc_p = INV8 if FP8_OUT[2] else 1.0

    # ACT: transcendentals straight out of PSUM, fp16 results in SBUF
    s_ = ew.tile([128, TC], F16, tag="s")
    nc.scalar.activation(s_[:], ph[:], ACT.Sigmoid, scale=sc_h)
    a_ = ew.tile([128, TC], F16, tag="a")
    nc.scalar.activation(a_[:], pg[:], ACT.Sigmoid, scale=-sc_g)
    g_ = ew.tile([128, TC], F16, tag="g")
    nc.scalar.activation(g_[:], pp[:], ACT.Sigmoid, scale=sc_p)
    # DVE: relu (PSUM read, 1x), then fp16 4x ops
    r_ = ew.tile([128, TC], F16, tag="r")
    if FP8_OUT[0]:
        nc.vector.tensor_scalar(r_[:], ph[:], sc_h, 0.0,
                                op0=ALU.mult, op1=ALU.max)
    else:
        nc.vector.tensor_scalar_max(r_[:], ph[:], 0.0)
    gh = ew.tile([128, TC], F16, tag="gh")
    nc.vector.scalar_tensor_tensor(gh[:], s_[:], 0.5, r_[:],
                                   op0=ALU.min, op1=ALU.add)
    negb = ew.tile([128, TC], F16, tag="nb")
    nc.vector.scalar_tensor_tensor(negb[:], a_[:], 1.0, gh[:],
                                   op0=ALU.subtract, op1=ALU.mult)
    sc = ew.tile([128, TC], F16, tag="sc")
    col = li * NFB + f
    init = 0.0 if i == 0 else carries[:, col:col + 1]
    nc.vector.tensor_tensor_scan(sc[:], a_[:], negb[:], init,
                                 op0=ALU.mult, op1=ALU.subtract)
    if i < NCHUNK - 1:
        nc.vector.tensor_copy(carries[:, col:col + 1], sc[:, TC - 1:TC])
    # highway: h' = h + g*(sc - h), into the next stage's rhs16 slice
    hs = rhs16[:, f, :]
    d_ = ew.tile([128, TC], F16, tag="d")
    nc.vector.scalar_tensor_tensor(d_[:], sc[:], 1.0, hs,
                                   op0=ALU.mult, op1=ALU.subtract)
    m_ = ew.tile([128, TC], F16, tag="m")
    nc.vector.scalar_tensor_tensor(m_[:], g_[:], 1.0, d_[:],
                                   op0=ALU.mult, op1=ALU.mult)
    nc.vector.scalar_tensor_tensor(dst16[:, f, :], m_[:], 1.0, hs,
                                   op0=ALU.mult, op1=ALU.add)
    if li == 0:
        if N8:
            nc.scalar.activation(dst8[:, f, :], dst16[:, f, :], ACT.Copy,
                                 scale=SH)
    else:
        nc.sync.dma_start(y16[f * 128:(f + 1) * 128, i * TC:(i + 1) * TC],
                          dst16[:, f, :])


def _emit_body(tc_, y16, h16t, h8t, w16_sb, w8_sb, pools):
    nc = tc_.nc
    rhs_pool, rhs8_pool, mm_psum, ew, carry_pool = pools
    carries = carry_pool.tile([128, 2 * NFB], F32)

    prev = None
    for i in range(NCHUNK):
        rhs16 = rhs_pool.tile([128, NK, TC], F16, tag="rhs16_l1")
        for k in range(NK):
            nc.sync.dma_start(rhs16[:, k, :],
                              h16t[k * 128:(k + 1) * 128, i * TC:(i + 1) * TC])
        rhs8 = None
        if N8:
            rhs8 = rhs8_pool.tile([128, NK, TC], F8, tag="rhs8_l1", name="rhs8")
            for k in range(NK):
                nc.sync.dma_start(rhs8[:, k, :],
                                  h8t[k * 128:(k + 1) * 128, i * TC:(i + 1) * TC])
        out16 = rhs_pool.tile([128, NK, TC], F16, tag="rhs16_l2")
        out8 = (rhs8_pool.tile([128, NK, TC], F8, tag="rhs8_l2", name="out8")
                if N8 else None)
        if prev is not None:
            # Layer-2 for chunk i-1 interleaved with layer-1 of chunk i:
            # emit f-block pairs so PE alternates between the two layers'
            # matmul groups while DVE/ACT drain the other's elementwise.
            (p16, p8, py) = prev
            for f in range(NFB):
                _emit_layer(nc, i, 0, f, w16_sb, w8_sb, rhs16, rhs8,
                            out16, out8, mm_psum, ew, carries, None)
                _emit_layer(nc, i - 1, 1, f, w16_sb, w8_sb, p16, p8,
                            py, None, mm_psum, ew, carries, y16)
        else:
            for f in range(NFB):
                _emit_layer(nc, i, 0, f, w16_sb, w8_sb, rhs16, rhs8,
                            out16, out8, mm_psum, ew, carries, None)
        ytile = rhs_pool.tile([128, NK, TC], F16, tag="y16")
        prev = (out16, out8, ytile)
    (p16, p8, py) = prev
    for f in range(NFB):
        _emit_layer(nc, NCHUNK - 1, 1, f, w16_sb, w8_sb, p16, p8,
                    py, None, mm_psum, ew, carries, y16)


def build_nc(loop_iters: int = 1):
    """Build + compile the per-core Bass program (SPMD across 8 cores)."""
    nc = bacc.Bacc("TRN2", target_bir_lowering=False, debug=False,
                   enable_asserts=False, num_devices=8)
    h16t = nc.dram_tensor("h16t", [H, T], F16, kind="ExternalInput").ap()
    h8t = (nc.dram_tensor("h8t", [H, T], F8, kind="ExternalInput").ap()
           if N8 else None)
    w16 = (nc.dram_tensor("w16", [2, NK, 128, C16], F16,
                          kind="ExternalInput").ap() if N16 else None)
    w8 = (nc.dram_tensor("w8", [2, NK, 128, C8], F8,
                         kind="ExternalInput").ap() if N8 else None)
    y16 = nc.dram_tensor("y16", [H, T], F16, kind="ExternalOutput").ap()

    with tile.TileContext(nc) as tc_:
        with ExitStack() as ctx:
            wpool = ctx.enter_context(tc_.tile_pool(name="w", bufs=1))
            rhs_pool = ctx.enter_context(tc_.tile_pool(name="rhs", bufs=2))
            rhs8_pool = ctx.enter_context(tc_.tile_pool(name="rhs8", bufs=2))
            mm_psum = ctx.enter_context(
                tc_.tile_pool(name="mmp", bufs=2, space="PSUM"))
            warm_psum = ctx.enter_context(
                tc_.tile_pool(name="warmp", bufs=1, space="PSUM"))
            ew = ctx.enter_context(tc_.tile_pool(name="ew", bufs=2))
            carry_pool = ctx.enter_context(tc_.tile_pool(name="carry", bufs=1))

            w16_sb = []
            w8_sb = []
            for li in range(2):
                if N16:
                    wt = wpool.tile([128, NK, C16], F16, tag=f"w16_{li}")
                    for k in range(NK):
                        nc.gpsimd.dma_start(wt[:, k, :], w16[li, k])
                    w16_sb.append(wt)
                if N8:
                    wt8 = wpool.tile([128, NK, C8], F8, tag=f"w8_{li}")
                    for k in range(NK):
                        nc.gpsimd.dma_start(wt8[:, k, :], w8[li, k])
                    w8_sb.append(wt8)

            # PE p-state warmup + ACT sigmoid table preload while the weight
            # stream is in flight: ~24 zero matmuls keep the PE busy through
            # the ramp window; the 1-element sigmoid forces the table load.
            warm_in = ew.tile([128, 2, TC], F16, tag="warm")
            nc.vector.memset(warm_in[:], 0.0)
            wp = warm_psum.tile([128, TC], F32, tag="warm")
            for _ in range(24):
                nc.tensor.matmul(wp[:], warm_in[:, 0, 0:128], warm_in[:, 0, :],
                                 start=True, stop=True)
            warm_s = ew.tile([128, TC], F16, tag="s")
            nc.scalar.activation(warm_s[:, 0:1], wp[:, 0:1], ACT.Sigmoid)

            pools = (rhs_pool, rhs8_pool, mm_psum, ew, carry_pool)
            if loop_iters == 1:
                _emit_body(tc_, y16, h16t, h8t, w16_sb, w8_sb, pools)
            else:
                with tc_.For_i(0, loop_iters, 1):
                    _emit_body(tc_, y16, h16t, h8t, w16_sb, w8_sb, pools)
    nc.compile()
    return nc


_CACHED_NC = None


def _prep_inputs(h, W0, W1):
    e4 = ml_dtypes.float8_e4m3
    W = np.stack([np.asarray(W0, np.float32), np.asarray(W1, np.float32)])
    cols16 = [W[:, :, o * H:(o + 1) * H] for o in range(3) if not FP8_OUT[o]]
    cols8 = [W[:, :, o * H:(o + 1) * H] for o in range(3) if FP8_OUT[o]]
    maps = []
    base = {}
    if N16:
        w16 = np.concatenate(cols16, axis=2).reshape(2, NK, 128, C16)
        base["w16"] = w16.astype(np.float16)
    if N8:
        w8 = (np.concatenate(cols8, axis=2) * SW).reshape(2, NK, 128, C8)
        base["w8"] = w8.astype(e4)
    for c in range(8):
        ht = np.ascontiguousarray(np.asarray(h[c]).T)
        m = dict(base)
        m["h16t"] = ht.astype(np.float16)
        if N8:
            m["h8t"] = (ht * SH).astype(e4)
        maps.append(m)
    return maps


def kernel(h, W0, W1):
    global _CACHED_NC
    if _CACHED_NC is None:
        _CACHED_NC = build_nc()
    res = bass_utils.run_bass_kernel_spmd(
        _CACHED_NC, _prep_inputs(h, W0, W1), core_ids=list(range(8)))
    return np.stack(
        [res.results[c]["y16"].T.astype(np.float32) for c in range(8)], axis=0)


# revision 4
# speedup vs baseline: 1.4980x; 1.4980x over previous
"""MinGRU (2-layer) Trainium2 Bass kernel, v3.

Problem: h[8,4096,1024] f32, W0/W1 [1024,3072] f32.
Per layer: z = h @ W; hidden,gate,proj = split(z);
  a = sigmoid(-gate); g_hidden = relu(hidden) + min(sigmoid(hidden), 0.5)
  scan: out_t = a_t*out_{t-1} + (1-a_t)*g_hidden_t   (fp32 scan state)
  h' = sigmoid(proj)*out + (1-sigmoid(proj))*h

Sharding: one batch row per core (B=8 over 8 cores), weights replicated.

Design (engine-balanced, measured op costs):
  - hidden matmul fp16 (accuracy-critical); gate/proj matmuls fp8 e4m3 with
    DoubleRow perf mode (2 k-tiles/instruction, 2x PE throughput).
    Measured rel err 1.28e-2 vs the 2e-2 gate, deterministic inputs.
  - host pre-transposes h to [H,T] fp16+fp8; y written [H,T] fp16 and
    host re-transposes; no PE transposes, no DMA transposes.
  - PSUM tiles span 2 banks [128,1024]; ACT reads them in one op
    (1.18us vs 2x0.91us) and writes fp16 SBUF.
  - elementwise spread across engines per 512-token subtile:
      ACT:  s=sig(hidden), r=relu(hidden), a=sig(-gate), g=sig(proj), fp8 cast
      DVE:  gh=min(s,.5)+r, negb=(a-1)*gh, tensor_tensor_scan, h'=m+h, carry
      Pool: d=sc-h, m=g*d
  - layer-2 runs one 1024-token span behind layer-1.
"""

import sys

if "/opt/trn_rl_repo" not in sys.path:
    sys.path.insert(0, "/opt/trn_rl_repo")

from contextlib import ExitStack

import numpy as np
import ml_dtypes

import concourse.bass as bass
import concourse.tile as tile
from concourse import bacc, mybir
from concourse import bass_utils

T, H, H3 = 4096, 1024, 3072
TCE = 1024               # elementwise span (= 2 PSUM banks of fp32)
TSUB = 512               # DVE/Pool subtile
NSPAN = T // TCE
NFB = H // 128           # output feature blocks
NK = H // 128            # contraction k-tiles
F32 = mybir.dt.float32
F16 = mybir.dt.float16
F8 = mybir.dt.float8e4
ACT = mybir.ActivationFunctionType
ALU = mybir.AluOpType
DR = mybir.MatmulPerfMode.DoubleRow

SH = 8.0                 # fp8 activation scale
SW = 32.0                # fp8 weight scale
INV8 = 1.0 / (SH * SW)
C16 = H                  # fp16 weight cols per layer (hidden)
C8 = 2 * H               # fp8 weight cols per layer (gate, proj)


def _emit_unit(nc, i, li, f, w16_sb, w8_sb, rhs16, rhs8, dst16, dst8,
               psums, ew, carries, y16):
    psum_h, psum_g, psum_p = psums
    ph = psum_h.tile([128, TCE], F32, tag="ph")
    pg = psum_g.tile([128, TCE], F32, tag="pg")
    pp = psum_p.tile([128, TCE], F32, tag="pp")
    w16 = w16_sb[li]
    w8 = w8_sb[li]
    for half in (0, 1):
        sl = slice(half * TSUB, (half + 1) * TSUB)
        for k in range(NK):
            nc.tensor.matmul(ph[:, sl], w16[:, k, f * 128:(f + 1) * 128],
                             rhs16[:, k, sl],
                             start=(k == 0), stop=(k == NK - 1))
        for k in range(0, NK, 2):
            nc.tensor.matmul(pg[:, sl], w8[:, k:k + 2, f * 128:(f + 1) * 128],
                             rhs8[:, k:k + 2, sl], perf_mode=DR,
                             start=(k == 0), stop=(k == NK - 2))
        for k in range(0, NK, 2):
            nc.tensor.matmul(pp[:, sl],
                             w8[:, k:k + 2, H + f * 128:H + (f + 1) * 128],
                             rhs8[:, k:k + 2, sl], perf_mode=DR,
                             start=(k == 0), stop=(k == NK - 2))
    # ACT: full-span transcendentals out of PSUM (s/r first: they gate the
    # next unit's hidden matmul group via psum reuse)
    s_ = ew.tile([128, TCE], F16, tag="s")
    nc.scalar.activation(s_[:], ph[:], ACT.Sigmoid)
    r_ = ew.tile([128, TCE], F16, tag="r")
    nc.scalar.activation(r_[:], ph[:], ACT.Relu)
    a_ = ew.tile([128, TCE], F16, tag="a")
    nc.scalar.activation(a_[:], pg[:], ACT.Sigmoid, scale=-INV8)
    g_ = ew.tile([128, TCE], F16, tag="g")
    nc.scalar.activation(g_[:], pp[:], ACT.Sigmoid, scale=INV8)

    col = li * NFB + f
    prev_sc = None
    for half in (0, 1):
        sl = slice(half * TSUB, (half + 1) * TSUB)
        gh = ew.tile([128, TSUB], F16, tag="gh")
        nc.vector.scalar_tensor_tensor(gh[:], s_[:, sl], 0.5, r_[:, sl],
                                       op0=ALU.min, op1=ALU.add)
        negb = ew.tile([128, TSUB], F16, tag="nb")
        nc.vector.scalar_tensor_tensor(negb[:], a_[:, sl], 1.0, gh[:],
                                       op0=ALU.subtract, op1=ALU.mult)
        sc = ew.tile([128, TSUB], F16, tag="sc")
        if half == 1:
            init = prev_sc[:, TSUB - 1:TSUB]
        elif i == 0:
            init = 0.0
        else:
            init = carries[:, col:col + 1]
        nc.vector.tensor_tensor_scan(sc[:], a_[:, sl], negb[:], init,
                                     op0=ALU.mult, op1=ALU.subtract)
        if half == 1 and i < NSPAN - 1:
            nc.vector.tensor_copy(carries[:, col:col + 1], sc[:, TSUB - 1:TSUB])
        prev_sc = sc
        hs = rhs16[:, f, sl]
        d_ = ew.tile([128, TSUB], F16, tag="d")
        nc.gpsimd.tensor_tensor(d_[:], sc[:], hs, op=ALU.subtract)
        m_ = ew.tile([128, TSUB], F16, tag="m")
        nc.gpsimd.tensor_tensor(m_[:], g_[:, sl], d_[:], op=ALU.mult)
        nc.vector.tensor_tensor(dst16[:, sl] if li else dst16[:, f, sl],
                                m_[:], hs, op=ALU.add)
    if li == 0:
        nc.scalar.activation(dst8[:, f, :], dst16[:, f, :], ACT.Copy, scale=SH)
    else:
        nc.sync.dma_start(y16[f * 128:(f + 1) * 128, i * TCE:(i + 1) * TCE],
                          dst16[:, :])


def _emit_body(tc_, y16, h16t, h8t, w16_sb, w8_sb, pools):
    nc = tc_.nc
    rhs_pool, ypool, psums, ew, carry_pool = pools
    carries = carry_pool.tile([128, 2 * NFB], F32)

    prev = None
    for i in range(NSPAN):
        rhs16 = rhs_pool.tile([128, NK, TCE], F16, tag="rhs16_l1")
        for k in range(NK):
            nc.sync.dma_start(rhs16[:, k, :],
                              h16t[k * 128:(k + 1) * 128, i * TCE:(i + 1) * TCE])
        rhs8 = rhs_pool.tile([128, NK, TCE], F8, tag="rhs8_l1")
        for k in range(NK):
            nc.sync.dma_start(rhs8[:, k, :],
                              h8t[k * 128:(k + 1) * 128, i * TCE:(i + 1) * TCE])
        out16 = rhs_pool.tile([128, NK, TCE], F16, tag="rhs16_l2")
        out8 = rhs_pool.tile([128, NK, TCE], F8, tag="rhs8_l2")
        for f in range(NFB):
            _emit_unit(nc, i, 0, f, w16_sb, w8_sb, rhs16, rhs8,
                       out16, out8, psums, ew, carries, None)
        if prev is not None:
            (p16, p8) = prev
            for f in range(NFB):
                ytile = ypool.tile([128, TCE], F16, tag="y", name="ytile")
                _emit_unit(nc, i - 1, 1, f, w16_sb, w8_sb, p16, p8,
                           ytile, None, psums, ew, carries, y16)
        prev = (out16, out8)
    (p16, p8) = prev
    for f in range(NFB):
        ytile = ypool.tile([128, TCE], F16, tag="y", name="ytile")
        _emit_unit(nc, NSPAN - 1, 1, f, w16_sb, w8_sb, p16, p8,
                   ytile, None, psums, ew, carries, y16)


def build_nc(loop_iters: int = 1):
    """Build + compile the per-core Bass program (SPMD across 8 cores)."""
    nc = bacc.Bacc("TRN2", target_bir_lowering=False, debug=False,
                   enable_asserts=False, num_devices=8)
    h16t = nc.dram_tensor("h16t", [H, T], F16, kind="ExternalInput").ap()
    h8t = nc.dram_tensor("h8t", [H, T], F8, kind="ExternalInput").ap()
    w16 = nc.dram_tensor("w16", [2, NK, 128, C16], F16,
                         kind="ExternalInput").ap()
    w8 = nc.dram_tensor("w8", [2, NK, 128, C8], F8,
                        kind="ExternalInput").ap()
    y16 = nc.dram_tensor("y16", [H, T], F16, kind="ExternalOutput").ap()

    with tile.TileContext(nc) as tc_:
        with ExitStack() as ctx:
            wpool = ctx.enter_context(tc_.tile_pool(name="w", bufs=1))
            rhs_pool = ctx.enter_context(tc_.tile_pool(name="rhs", bufs=2))
            ypool = ctx.enter_context(tc_.tile_pool(name="y", bufs=2))
            psum_h = ctx.enter_context(
                tc_.tile_pool(name="psh", bufs=2, space="PSUM"))
            psum_g = ctx.enter_context(
                tc_.tile_pool(name="psg", bufs=1, space="PSUM"))
            psum_p = ctx.enter_context(
                tc_.tile_pool(name="psp", bufs=1, space="PSUM"))
            ew = ctx.enter_context(tc_.tile_pool(name="ew", bufs=2))
            carry_pool = ctx.enter_context(tc_.tile_pool(name="carry", bufs=1))

            w16_sb = []
            w8_sb = []
            for li in range(2):
                wt = wpool.tile([128, NK, C16], F16, tag=f"w16_{li}",
                                name=f"w16_{li}")
                for k in range(NK):
                    nc.gpsimd.dma_start(wt[:, k, :], w16[li, k])
                w16_sb.append(wt)
                wt8 = wpool.tile([128, NK, C8], F8, tag=f"w8_{li}",
                                 name=f"w8_{li}")
                for k in range(NK):
                    nc.gpsimd.dma_start(wt8[:, k, :], w8[li, k])
                w8_sb.append(wt8)

            # PE p-state warmup + ACT sigmoid table preload while the weight
            # stream is in flight. The warm matmuls write the proj psum tile
            # (reused by the first real unit afterwards).
            warm_in = ew.tile([128, TSUB], F16, tag="warm")
            nc.vector.memset(warm_in[:], 0.0)
            wp = psum_p.tile([128, TCE], F32, tag="pp", name="wp")
            for _ in range(24):
                nc.tensor.matmul(wp[:, 0:TSUB], warm_in[:, 0:128],
                                 warm_in[:], start=True, stop=True)
            warm_s = ew.tile([128, TCE], F16, tag="s", name="warm_s")
            nc.scalar.activation(warm_s[:, 0:1], wp[:, 0:1], ACT.Sigmoid)

            pools = (rhs_pool, ypool, (psum_h, psum_g, psum_p), ew, carry_pool)
            if loop_iters == 1:
                _emit_body(tc_, y16, h16t, h8t, w16_sb, w8_sb, pools)
            else:
                with tc_.For_i(0, loop_iters, 1):
                    _emit_body(tc_, y16, h16t, h8t, w16_sb, w8_sb, pools)
    nc.compile()
    return nc


_CACHED_NC = None


def _prep_inputs(h, W0, W1):
    e4 = ml_dtypes.float8_e4m3
    W = np.stack([np.asarray(W0, np.float32), np.asarray(W1, np.float32)])
    w16 = W[:, :, 0:H].reshape(2, NK, 128, C16)
    w8 = (W[:, :, H:] * SW).reshape(2, NK, 128, C8)
    base = {"w16": w16.astype(np.float16), "w8": w8.astype(e4)}
    maps = []
    for c in range(8):
        ht = np.ascontiguousarray(np.asarray(h[c]).T)
        m = dict(base)
        m["h16t"] = ht.astype(np.float16)
        m["h8t"] = (ht * SH).astype(e4)
        maps.append(m)
    return maps


def kernel(h, W0, W1):
    global _CACHED_NC
    if _CACHED_NC is None:
        _CACHED_NC = build_nc()
    res = bass_utils.run_bass_kernel_spmd(
        _CACHED_NC, _prep_inputs(h, W0, W1), core_ids=list(range(8)))
    return np.stack(
        [res.results[c]["y16"].T.astype(np.float32) for c in range(8)], axis=0)


# revision 7
# speedup vs baseline: 1.6660x; 1.1121x over previous
"""MinGRU (2-layer) Trainium2 Bass kernel, v3.

Problem: h[8,4096,1024] f32, W0/W1 [1024,3072] f32.
Per layer: z = h @ W; hidden,gate,proj = split(z);
  a = sigmoid(-gate); g_hidden = relu(hidden) + min(sigmoid(hidden), 0.5)
  scan: out_t = a_t*out_{t-1} + (1-a_t)*g_hidden_t   (fp32 scan state)
  h' = sigmoid(proj)*out + (1-sigmoid(proj))*h

Sharding: one batch row per core (B=8 over 8 cores), weights replicated.

Design (engine-balanced, measured op costs):
  - hidden matmul fp16 (accuracy-critical); gate/proj matmuls fp8 e4m3 with
    DoubleRow perf mode (2 k-tiles/instruction, 2x PE throughput).
    Measured rel err 1.28e-2 vs the 2e-2 gate, deterministic inputs.
  - host pre-transposes h to [H,T] fp16+fp8; y written [H,T] fp16 and
    host re-transposes; no PE transposes, no DMA transposes.
  - PSUM tiles span 2 banks [128,1024]; ACT reads them in one op
    (1.18us vs 2x0.91us) and writes fp16 SBUF.
  - elementwise spread across engines per 512-token subtile:
      ACT:  s=sig(hidden), r=relu(hidden), a=sig(-gate), g=sig(proj), fp8 cast
      DVE:  gh=min(s,.5)+r, negb=(a-1)*gh, tensor_tensor_scan, h'=m+h, carry
      Pool: d=sc-h, m=g*d
  - layer-2 runs one 1024-token span behind layer-1.
"""

import sys

if "/opt/trn_rl_repo" not in sys.path:
    sys.path.insert(0, "/opt/trn_rl_repo")

from contextlib import ExitStack

import numpy as np
import ml_dtypes

import concourse.bass as bass
import concourse.tile as tile
from concourse import bacc, mybir
from concourse import bass_utils

T, H, H3 = 4096, 1024, 3072
TCE = 1024               # elementwise span (= 2 PSUM banks of fp32)
TSUB = 512               # DVE/Pool subtile
NSPAN = T // TCE
NFB = H // 128           # output feature blocks
NK = H // 128            # contraction k-tiles
F32 = mybir.dt.float32
F16 = mybir.dt.float16
F8 = mybir.dt.float8e4
ACT = mybir.ActivationFunctionType
ALU = mybir.AluOpType
DR = mybir.MatmulPerfMode.DoubleRow

SH = 8.0                 # fp8 activation scale
SW = 32.0                # fp8 weight scale
INV8 = 1.0 / (SH * SW)
C16 = H                  # fp16 weight cols per layer (hidden)
C8 = 2 * H               # fp8 weight cols per layer (gate, proj)


def _emit_unit(nc, i, li, f, w16_sb, w8_sb, rhs16, rhs8, dst16, dst8,
               psums, ew, carries, y16):
    """Emit matmuls + front elementwise for one (span, layer, f-block).

    Returns a closure emitting the tail (DVE highway-out, then ACT fp8 cast
    or the y DMA) which the caller schedules 1-2 units later so the in-order
    ACT/DVE streams never block on the cross-engine scan->Pool chain.
    """
    psum_h, psum_g, psum_p = psums
    ph = psum_h.tile([128, TCE], F32, tag="ph")
    pg = psum_g.tile([128, TCE], F32, tag="pg")
    pp = psum_p.tile([128, TCE], F32, tag="pp")
    w16 = w16_sb[li]
    w8 = w8_sb[li]
    for half in (0, 1):
        sl = slice(half * TSUB, (half + 1) * TSUB)
        for k in range(NK):
            nc.tensor.matmul(ph[:, sl], w16[:, k, f * 128:(f + 1) * 128],
                             rhs16[:, k, sl],
                             start=(k == 0), stop=(k == NK - 1))
        for k in range(0, NK, 2):
            nc.tensor.matmul(pg[:, sl], w8[:, k:k + 2, f * 128:(f + 1) * 128],
                             rhs8[:, k:k + 2, sl], perf_mode=DR,
                             start=(k == 0), stop=(k == NK - 2))
        for k in range(0, NK, 2):
            nc.tensor.matmul(pp[:, sl],
                             w8[:, k:k + 2, H + f * 128:H + (f + 1) * 128],
                             rhs8[:, k:k + 2, sl], perf_mode=DR,
                             start=(k == 0), stop=(k == NK - 2))
    # ACT: full-span transcendentals out of PSUM (s/r first: they gate the
    # next unit's hidden matmul group via psum reuse)
    s_ = ew.tile([128, TCE], F16, tag="s")
    nc.scalar.activation(s_[:], ph[:], ACT.Sigmoid)
    r_ = ew.tile([128, TCE], F16, tag="r")
    nc.scalar.activation(r_[:], ph[:], ACT.Relu)
    a_ = ew.tile([128, TCE], F16, tag="a")
    nc.scalar.activation(a_[:], pg[:], ACT.Sigmoid, scale=-INV8)
    g_ = ew.tile([128, TCE], F16, tag="g")
    nc.scalar.activation(g_[:], pp[:], ACT.Sigmoid, scale=INV8)

    col = li * NFB + f
    prev_sc = None
    ms = []
    for half in (0, 1):
        sl = slice(half * TSUB, (half + 1) * TSUB)
        gh = ew.tile([128, TSUB], F16, tag="gh")
        nc.vector.scalar_tensor_tensor(gh[:], s_[:, sl], 0.5, r_[:, sl],
                                       op0=ALU.min, op1=ALU.add)
        negb = ew.tile([128, TSUB], F16, tag="nb")
        nc.vector.scalar_tensor_tensor(negb[:], a_[:, sl], 1.0, gh[:],
                                       op0=ALU.subtract, op1=ALU.mult)
        sc = ew.tile([128, TSUB], F16, tag="sc")
        if half == 1:
            init = prev_sc[:, TSUB - 1:TSUB]
        elif i == 0:
            init = 0.0
        else:
            init = carries[:, col:col + 1]
        nc.vector.tensor_tensor_scan(sc[:], a_[:, sl], negb[:], init,
                                     op0=ALU.mult, op1=ALU.subtract)
        if half == 1 and i < NSPAN - 1:
            nc.vector.tensor_copy(carries[:, col:col + 1], sc[:, TSUB - 1:TSUB])
        prev_sc = sc
        hs = rhs16[:, f, sl]
        d_ = ew.tile([128, TSUB], F16, tag="d")
        nc.gpsimd.tensor_tensor(d_[:], sc[:], hs, op=ALU.subtract)
        m_ = ew.tile([128, TSUB], F16, tag="m")
        nc.gpsimd.tensor_tensor(m_[:], g_[:, sl], d_[:], op=ALU.mult)
        ms.append(m_)

    def tail_dve():
        for half in (0, 1):
            sl = slice(half * TSUB, (half + 1) * TSUB)
            nc.vector.tensor_tensor(dst16[:, sl] if li else dst16[:, f, sl],
                                    ms[half][:], rhs16[:, f, sl], op=ALU.add)

    def tail_fin():
        if li == 0:
            nc.scalar.activation(dst8[:, f, :], dst16[:, f, :], ACT.Copy,
                                 scale=SH)
        else:
            nc.sync.dma_start(
                y16[f * 128:(f + 1) * 128, i * TCE:(i + 1) * TCE],
                dst16[:, :])

    return tail_dve, tail_fin


def _emit_body(tc_, y16, h16t, h8t, w16_sb, w8_sb, pools):
    nc = tc_.nc
    rhs_pool, ypool, psums, ew, carry_pool = pools
    carries = carry_pool.tile([128, 2 * NFB], F32)

    # Software-pipelined tails: DVE highway-out runs 1 unit behind its
    # producer, the ACT cast / y-DMA 2 units behind.
    pend_dve = []
    pend_fin = []

    def emit(unit_args):
        # pop delayed tails first: their reads must precede the new front's
        # buffer-rotating writes in emission order
        if pend_dve:
            pend_dve.pop(0)()
        if len(pend_fin) >= 2:
            pend_fin.pop(0)()
        tail_dve, tail_fin = _emit_unit(*unit_args)
        pend_dve.append(tail_dve)
        pend_fin.append(tail_fin)

    prev = None
    for i in range(NSPAN):
        rhs16 = rhs_pool.tile([128, NK, TCE], F16, tag="rhs16_l1")
        for k in range(NK):
            nc.sync.dma_start(rhs16[:, k, :],
                              h16t[k * 128:(k + 1) * 128, i * TCE:(i + 1) * TCE])
        rhs8 = rhs_pool.tile([128, NK, TCE], F8, tag="rhs8_l1")
        for k in range(NK):
            nc.sync.dma_start(rhs8[:, k, :],
                              h8t[k * 128:(k + 1) * 128, i * TCE:(i + 1) * TCE])
        out16 = rhs_pool.tile([128, NK, TCE], F16, tag="rhs16_l2")
        out8 = rhs_pool.tile([128, NK, TCE], F8, tag="rhs8_l2")
        if prev is None:
            for f in range(NFB):
                emit((nc, i, 0, f, w16_sb, w8_sb, rhs16, rhs8,
                      out16, out8, psums, ew, carries, None))
            # span 0 has no interleaved L2 units; flush so span 1's L2
            # matmuls see every span-0 cast already emitted
            for t in pend_dve:
                t()
            for t in pend_fin:
                t()
            pend_dve.clear()
            pend_fin.clear()
        else:
            (p16, p8) = prev
            for f in range(NFB):
                emit((nc, i, 0, f, w16_sb, w8_sb, rhs16, rhs8,
                      out16, out8, psums, ew, carries, None))
                ytile = ypool.tile([128, TCE], F16, tag="y", name="ytile")
                emit((nc, i - 1, 1, f, w16_sb, w8_sb, p16, p8,
                      ytile, None, psums, ew, carries, y16))
        prev = (out16, out8)
    (p16, p8) = prev
    for f in range(NFB):
        ytile = ypool.tile([128, TCE], F16, tag="y", name="ytile")
        emit((nc, NSPAN - 1, 1, f, w16_sb, w8_sb, p16, p8,
              ytile, None, psums, ew, carries, y16))
    for t in pend_dve:
        t()
    for t in pend_fin:
        t()


def build_nc(loop_iters: int = 1):
    """Build + compile the per-core Bass program (SPMD across 8 cores)."""
    nc = bacc.Bacc("TRN2", target_bir_lowering=False, debug=False,
                   enable_asserts=False, num_devices=8)
    h16t = nc.dram_tensor("h16t", [H, T], F16, kind="ExternalInput").ap()
    h8t = nc.dram_tensor("h8t", [H, T], F8, kind="ExternalInput").ap()
    w16 = nc.dram_tensor("w16", [2, NK, 128, C16], F16,
                         kind="ExternalInput").ap()
    w8 = nc.dram_tensor("w8", [2, NK, 128, C8], F8,
                        kind="ExternalInput").ap()
    y16 = nc.dram_tensor("y16", [H, T], F16, kind="ExternalOutput").ap()

    with tile.TileContext(nc) as tc_:
        with ExitStack() as ctx:
            wpool = ctx.enter_context(tc_.tile_pool(name="w", bufs=1))
            rhs_pool = ctx.enter_context(tc_.tile_pool(name="rhs", bufs=2))
            ypool = ctx.enter_context(tc_.tile_pool(name="y", bufs=2))
            psum_h = ctx.enter_context(
                tc_.tile_pool(name="psh", bufs=2, space="PSUM"))
            psum_g = ctx.enter_context(
                tc_.tile_pool(name="psg", bufs=1, space="PSUM"))
            psum_p = ctx.enter_context(
                tc_.tile_pool(name="psp", bufs=1, space="PSUM"))
            ew = ctx.enter_context(tc_.tile_pool(name="ew", bufs=2))
            carry_pool = ctx.enter_context(tc_.tile_pool(name="carry", bufs=1))

            w16_sb = []
            w8_sb = []
            for li in range(2):
                wt = wpool.tile([128, NK, C16], F16, tag=f"w16_{li}",
                                name=f"w16_{li}")
                for k in range(NK):
                    nc.gpsimd.dma_start(wt[:, k, :], w16[li, k])
                w16_sb.append(wt)
                wt8 = wpool.tile([128, NK, C8], F8, tag=f"w8_{li}",
                                 name=f"w8_{li}")
                for k in range(NK):
                    nc.gpsimd.dma_start(wt8[:, k, :], w8[li, k])
                w8_sb.append(wt8)

            # PE p-state warmup + ACT sigmoid table preload while the weight
            # stream is in flight. The warm matmuls write the proj psum tile
            # (reused by the first real unit afterwards).
            warm_in = ew.tile([128, TSUB], F16, tag="warm")
            nc.vector.memset(warm_in[:], 0.0)
            wp = psum_p.tile([128, TCE], F32, tag="pp", name="wp")
            for _ in range(24):
                nc.tensor.matmul(wp[:, 0:TSUB], warm_in[:, 0:128],
                                 warm_in[:], start=True, stop=True)
            warm_s = ew.tile([128, TCE], F16, tag="s", name="warm_s")
            nc.scalar.activation(warm_s[:, 0:1], wp[:, 0:1], ACT.Sigmoid)

            pools = (rhs_pool, ypool, (psum_h, psum_g, psum_p), ew, carry_pool)
            if loop_iters == 1:
                _emit_body(tc_, y16, h16t, h8t, w16_sb, w8_sb, pools)
            else:
                with tc_.For_i(0, loop_iters, 1):
                    _emit_body(tc_, y16, h16t, h8t, w16_sb, w8_sb, pools)
    nc.compile()
    return nc


_CACHED_NC = None


def _prep_inputs(h, W0, W1):
    e4 = ml_dtypes.float8_e4m3
    W = np.stack([np.asarray(W0, np.float32), np.asarray(W1, np.float32)])
    w16 = W[:, :, 0:H].reshape(2, NK, 128, C16)
    w8 = (W[:, :, H:] * SW).reshape(2, NK, 128, C8)
    base = {"w16": w16.astype(np.float16), "w8": w8.astype(e4)}
    maps = []
    for c in range(8):
        ht = np.ascontiguousarray(np.asarray(h[c]).T)
        m = dict(base)
        m["h16t"] = ht.astype(np.float16)
        m["h8t"] = (ht * SH).astype(e4)
        maps.append(m)
    return maps


def kernel(h, W0, W1):
    global _CACHED_NC
    if _CACHED_NC is None:
        _CACHED_NC = build_nc()
    res = bass_utils.run_bass_kernel_spmd(
        _CACHED_NC, _prep_inputs(h, W0, W1), core_ids=list(range(8)))
    return np.stack(
        [res.results[c]["y16"].T.astype(np.float32) for c in range(8)], axis=0)
